# revision 26
# baseline (speedup 1.0000x reference)
"""Trainium2 Bass kernel for CNN-BiLSTM encoder/decoder (nn_CNN_BiLSTM_AttenQ).

Data-parallel over batch: B=128 sharded 8 ways (16 samples/core), weights
replicated, no collectives.

v2 design (vs baseline):
  recurrence: per stream-step cell is 1 ACT + 2 DVE + 1 add (gpsimd/dve):
    PSUM gates per step [f|i|g|o] (f,i,o pre-scaled by 0.25, +0.5 bias so
    clamp01 == hard-sigmoid). ACT relu-copies g into CG odd slots; one
    fused DVE op (FCM) computes (min(relu(f),1)*c, min(relu(i),1)*g)
    in-place into CG; add folds CG even+odd -> c'; HSIG_MUL produces h.
  U/W gate blocks are separate contiguous [128,128] tiles (FWL weight loads),
  one weight load shared by both batch streams.
  decoder: stage-major over groups of 4 samples; 32-channel convs packed
  block-diagonally (d21/d22: 4 samples, up2: 2 samples) to fill the PE
  array; PSUM->SBUF copies batched into single wide ACT ops.
"""

import os
import numpy as np
import ml_dtypes

import bass_rust
import concourse.mybir as mybir
import concourse.tile as tile
from concourse import bacc
from concourse import dve_ops
from concourse.dve_spec import (Spec, Src0, Src1, C0, C1, One, relu, minn,
                                lower, _has_src1)
from concourse.dve_uop import DveOpSpec
from concourse.bass_utils import run_bass_kernel_spmd
from contextlib import ExitStack


def _register_dve_op(name, body, ref):
    for op in dve_ops.OPS:
        if op.name == name:
            return op
    spec = Spec(body=body, reference=ref)
    op = dve_ops.DveOp(name, spec, subdim=False, uops_sha={})
    dve_ops._SUB_OPCODE_FOR_NAME[name] = max(dve_ops._SUB_OPCODE_FOR_NAME.values()) + 1
    dve_ops.OPS.append(op)
    dve_ops.CUSTOM_DVE_SPECS[name] = spec
    for ver in ("v3", "v4"):
        uops = lower(spec, ver=ver)
        op.uops_sha[ver] = DveOpSpec(
            name=name, opcode=dve_ops.get_dve_sub_opcode(name), uops=uops,
            rd1_en=_has_src1(spec)).sha(ver)
    return op


# out = min(relu(in0),1) * relu(in1)   (pairs (f,c),(i,g))
FCM = _register_dve_op(
    "ANT_FCM", minn(relu(Src0), One) * relu(Src1),
    lambda in0, in1, s0, s1, imm2: np.minimum(np.maximum(in0, 0), 1)
    * np.maximum(in1, 0))
# out = clamp01(in0*s0 + s1) * clamp01(in1)
HSIG_MUL = _register_dve_op(
    "ANT_HSIG_MUL",
    minn(relu(Src0 * C0 + C1), One) * minn(relu(Src1), One),
    lambda in0, in1, s0, s1, imm2: np.minimum(np.maximum(in0 * s0 + s1, 0), 1)
    * np.minimum(np.maximum(in1, 0), 1))

F32 = mybir.dt.float32
BF16 = mybir.dt.bfloat16
AF = mybir.ActivationFunctionType
OP = mybir.AluOpType
BFNP = ml_dtypes.bfloat16

B, T, C, HS = 128, 2048, 128, 128
N_CORES = 8
BL = B // N_CORES          # 16 samples per core
L = T // 4                 # 512 encoder output length
CH = 32                    # recurrence chunk length (timesteps)
NCH = L // CH
L1 = 2 * L                 # 1024
L2 = T                     # 2048
PAD = 4                    # halo pad per batch segment in decoder buffers
EPS = 1e-5
NS = 2                     # phase-offset streams per core
SB = BL // NS              # 8 samples per stream

# convT taps: out[2m+r] += x[m+delta] @ w[:, :, k]  -> list of (delta, k)
CONVT_TAPS = {0: [(1, 1), (0, 3), (-1, 5), (-2, 7)],
              1: [(2, 0), (1, 2), (0, 4), (-1, 6)]}
K5_TAPS = [(k - 2, k) for k in range(5)]

ADD_ENGINE = os.environ.get("BASS_ADD_ENGINE", "gp")


def _bf(x):
    return np.ascontiguousarray(np.asarray(x, np.float32).astype(BFNP))


def _f32(x):
    return np.ascontiguousarray(np.asarray(x, np.float32))


def _prep_dir(W, U, b):
    # torch gate order [i,f,g,o] -> v2 order [f,i,g,o]
    perm = [1, 0, 2, 3]
    Wp = np.concatenate([W[:, j * HS:(j + 1) * HS] for j in perm], 1).astype(np.float64)
    Up = np.concatenate([U[:, j * HS:(j + 1) * HS] for j in perm], 1).astype(np.float64)
    bp = np.concatenate([b[j * HS:(j + 1) * HS] for j in perm]).astype(np.float64)
    # f,i,o scaled 0.25 (+0.5 bias); g raw (relu)
    for j in (0, 1, 3):
        Wp[:, j * HS:(j + 1) * HS] *= 0.25
        Up[:, j * HS:(j + 1) * HS] *= 0.25
        bp[j * HS:(j + 1) * HS] = 0.25 * bp[j * HS:(j + 1) * HS] + 0.5
    bA = np.stack([bp[0:HS], bp[HS:2 * HS]])          # (b_f, b_i)
    bB = np.stack([bp[2 * HS:3 * HS], bp[3 * HS:]])   # (b_g, b_o)
    return Wp, Up, _bf(bA), _bf(bB)


def _prep_weights(inp):
    w = {}
    w["w1"] = _bf(np.asarray(inp["conv1_w"])[:, 0, :].T)            # [8,128]
    w["s1"] = _f32(np.asarray(inp["bn1_g"]) / np.sqrt(1.0 + EPS))[:, None]
    w["b1"] = _f32(np.asarray(inp["bn1_b"]))[:, None]
    for tag, (Wk, Uk, bk) in {
        "0f": ("W0f", "U0f", "b0f"), "0r": ("W0r", "U0r", "b0r"),
        "1f": ("W1f", "U1f", "b1f"), "1r": ("W1r", "U1r", "b1r"),
    }.items():
        Wp, Up, bA, bB = _prep_dir(np.asarray(inp[Wk], np.float32),
                                   np.asarray(inp[Uk], np.float32),
                                   np.asarray(inp[bk], np.float32))
        for j in range(4):
            w[f"U{tag}_{j}"] = _bf(Up[:, j * HS:(j + 1) * HS])
            w[f"Wa{tag}_{j}"] = _bf(Wp[:HS, j * HS:(j + 1) * HS])
            if Wp.shape[0] == 2 * HS:
                w[f"Wb{tag}_{j}"] = _bf(Wp[HS:, j * HS:(j + 1) * HS])
        w[f"bA{tag}"], w[f"bB{tag}"] = bA, bB
    # up1: 16 blocks [128,128]: blk = (r*4+ti)*2+ch
    u1w = np.asarray(inp["up1_w"], np.float32)                      # [256,128,8]
    for r in (0, 1):
        for ti, (_, k) in enumerate(CONVT_TAPS[r]):
            for ch in (0, 1):
                blk = (r * 4 + ti) * 2 + ch
                w[f"u1_{blk}"] = _bf(u1w[ch * HS:(ch + 1) * HS, :, k])
    d11w = np.asarray(inp["d11_w"], np.float32)
    for k in range(5):
        w[f"d11_{k}"] = _bf(d11w[:, :, k].T)
    w["s11"] = _f32(np.asarray(inp["bn11_g"]) / np.sqrt(1.0 + EPS))[:, None]
    w["b11"] = _f32(np.asarray(inp["bn11_b"]))[:, None]
    d12w = np.asarray(inp["d12_w"], np.float32)
    for k in range(5):
        w[f"d12_{k}"] = _bf(d12w[:, :, k].T)
    # replicated x2 so partition-base-64 slices stay aligned
    w["s12"] = _f32(np.tile(np.asarray(inp["bn12_g"]) / np.sqrt(1.0 + EPS), 2))[:, None]
    w["b12"] = _f32(np.tile(np.asarray(inp["bn12_b"]), 2))[:, None]
    # up2: block-diag 2-sample [128, 64] per (r, ti)
    u2w = np.asarray(inp["up2_w"], np.float32)                      # [64,32,8]
    for r in (0, 1):
        for ti, (_, k) in enumerate(CONVT_TAPS[r]):
            bd = np.zeros((128, 64), np.float32)
            for s in (0, 1):
                bd[s * 64:(s + 1) * 64, s * 32:(s + 1) * 32] = u2w[:, :, k]
            w[f"u2_{r}{ti}"] = _bf(bd)
    # d21: block-diag 4-sample [128,128] per tap
    d21w = np.asarray(inp["d21_w"], np.float32)
    for k in range(5):
        bd = np.zeros((128, 128), np.float32)
        for s in range(4):
            bd[s * 32:(s + 1) * 32, s * 32:(s + 1) * 32] = d21w[:, :, k].T
        w[f"d21_{k}"] = _bf(bd)
    w["s21"] = _f32(np.tile(np.asarray(inp["bn21_g"]) / np.sqrt(1.0 + EPS), 4))[:, None]
    w["b21"] = _f32(np.tile(np.asarray(inp["bn21_b"]), 4))[:, None]
    # d22: block-diag 4-sample [128,16] per tap
    d22w = np.asarray(inp["d22_w"], np.float32)
    for k in range(5):
        bd = np.zeros((128, 16), np.float32)
        for s in range(4):
            bd[s * 32:(s + 1) * 32, s * 4:(s + 1) * 4] = d22w[:, :, k].T
        w[f"d22_{k}"] = _bf(bd)
    w["s22"] = _f32(np.tile(np.asarray(inp["bn22_g"]) / np.sqrt(1.0 + EPS), 4))[:, None]
    w["b22"] = _f32(np.tile(np.asarray(inp["bn22_b"]), 4))[:, None]
    # bias-spread indicator [2, 512]: gate-major: row0 on first 256 cols
    blk = CH * SB
    ind = np.zeros((2, 2 * blk), np.float32)
    ind[0, :blk] = 1.0
    ind[1, blk:] = 1.0
    w["ind"] = _bf(ind)
    return w


def _prep_xcol(x_shard):
    xp = np.pad(np.asarray(x_shard, np.float32)[:, 0, :], ((0, 0), (3, 4)))
    cols = np.stack([xp[:, k:k + 4 * L:4] for k in range(8)])       # [8,BL,L]
    return _bf(cols.reshape(8, BL * L))


def _wspec():
    spec = [
        ("w1", [8, HS], BF16), ("s1", [HS, 1], F32), ("b1", [HS, 1], F32),
        ("ind", [2, CH * 2 * SB], BF16),
    ]
    for tag in ("0f", "0r", "1f", "1r"):
        for j in range(4):
            spec.append((f"U{tag}_{j}", [HS, HS], BF16))
            spec.append((f"Wa{tag}_{j}", [HS, HS], BF16))
            if tag.startswith("1"):
                spec.append((f"Wb{tag}_{j}", [HS, HS], BF16))
        spec.append((f"bA{tag}", [2, HS], BF16))
        spec.append((f"bB{tag}", [2, HS], BF16))
    for blk in range(16):
        spec.append((f"u1_{blk}", [HS, HS], BF16))
    for k in range(5):
        spec.append((f"d11_{k}", [HS, HS], BF16))
        spec.append((f"d12_{k}", [HS, 64], BF16))
        spec.append((f"d21_{k}", [HS, HS], BF16))
        spec.append((f"d22_{k}", [HS, 16], BF16))
    for r in (0, 1):
        for ti in range(4):
            spec.append((f"u2_{r}{ti}", [HS, 64], BF16))
    spec += [("s11", [HS, 1], F32), ("b11", [HS, 1], F32),
             ("s12", [HS, 1], F32), ("b12", [HS, 1], F32),
             ("s21", [HS, 1], F32), ("b21", [HS, 1], F32),
             ("s22", [16, 1], F32), ("b22", [16, 1], F32)]
    return spec


_WSPEC = _wspec()


def _emit_pass(nc, pools, wt, ins, Hout, h_aps, cg_tiles, reverse, t_off):
    """One LSTM direction (L steps, NCH chunks), NS streams.
    ins: input APs [128,BL,L]. Hout: [128,BL,Lseg]; h at [:,bs,t_off+t].
    cg_tiles[si]: [128, 8, 2] view-able CG state (even=c, odd=g-scratch)."""
    psum_pool = pools["psum_r"]
    ind = pools["ind"]
    h_aps = list(h_aps)

    def chunk_pieces(ci, si):
        """Allocate pg for chunk ci of stream si; return (pg, T0, pieces).
        pg gate-major [HS, 4(gates f,i,g,o), CH, SB]: every matmul out is a
        contiguous single-bank region."""
        T0 = ci * CH if not reverse else L - (ci + 1) * CH
        pg = psum_pool.tile([HS, 4, CH, SB], F32, tag=f"pg{si}", name=f"pg{si}")

        def bias():
            nc.tensor.matmul(pg[:, 0:2, :, :], wt["bA"][:], ind[:],
                             start=True, stop=False)
            nc.tensor.matmul(pg[:, 2:4, :, :], wt["bB"][:], ind[:],
                             start=True, stop=False)

        pieces = [bias]
        for j in range(4):
            def wmm(j=j):
                Wlists = [wt["Wa"]] + ([wt["Wb"]] if "Wb" in wt else [])
                for idx, Wl in enumerate(Wlists):
                    rhs = ins[idx].rearrange("p b t -> p t b")[
                        :, T0:T0 + CH, si * SB:(si + 1) * SB]
                    nc.tensor.matmul(pg[:, j, :, :], Wl[j][:], rhs,
                                     start=False, stop=False)
            pieces.append(wmm)
        return pg, T0, pieces

    nxt = [chunk_pieces(0, si) for si in range(NS)]
    for _, _, pieces in nxt:
        for p in pieces:
            p()
    for ci in range(NCH):
        pgs = [pg for pg, _, _ in nxt]
        T0 = nxt[0][1]
        if ci + 1 < NCH:
            nxt = [chunk_pieces(ci + 1, si) for si in range(NS)]
            todo = [p for _, _, pieces in nxt for p in pieces]
        else:
            nxt, todo = [], []
        n_todo, done = len(todo), 0
        for s in range(CH):
            tl = s if not reverse else CH - 1 - s
            t = T0 + tl
            # U matmuls: j outer so both streams share each weight load;
            # g-copies issue right after the g-gate MMs so both streams'
            # ACT work is in flight before the DVE cells need it
            for j in range(3):
                for si in range(NS):
                    nc.tensor.matmul(pgs[si][:, j, tl, :],
                                     wt["U"][j][:], h_aps[si],
                                     start=False, stop=False)
            for si in range(NS):
                nc.scalar.activation(cg_tiles[si][:, :, 1],
                                     pgs[si][:, 2, tl, :], AF.Relu)
            for si in range(NS):
                nc.tensor.matmul(pgs[si][:, 3, tl, :],
                                 wt["U"][3][:], h_aps[si],
                                 start=False, stop=True)
            for si in range(NS):
                pg, cgq = pgs[si], cg_tiles[si]
                # (f,c),(i,g) -> T = (f^*c, i^*relu(g))
                tt = pools["tmp"].tile([HS, SB, 2], F32, tag=f"t{si}", name="tt")
                in0 = pg[:, 0:2, tl, :].rearrange("p q s -> p s q")
                nc.vector._custom_dve(FCM, out=tt[:], in0=in0, in1=cgq[:])
                # c' = cf + t1
                if ADD_ENGINE == "gp":
                    nc.gpsimd.tensor_tensor(cgq[:, :, 0], tt[:, :, 0],
                                            tt[:, :, 1], OP.add)
                else:
                    nc.vector.tensor_tensor(cgq[:, :, 0], tt[:, :, 0],
                                            tt[:, :, 1], OP.add)
                # h = clamp01(0.25c+0.5)*clamp01(o)
                h_aps[si] = Hout[:, si * SB:(si + 1) * SB, t_off + t]
                nc.vector._custom_dve(HSIG_MUL, out=h_aps[si],
                                      in0=cgq[:, :, 0], in1=pg[:, 3, tl, :],
                                      s0=0.25, s1=0.5)
            # spread next-chunk precompute pieces evenly across steps
            while todo and done < (s + 1) * n_todo // CH:
                todo.pop(0)()
                done += 1
    return h_aps


DEBUG_OUT = bool(int(os.environ.get("BASS_DEBUG_OUT", "0")))


def build_nc():
    nc = bacc.Bacc()
    xcol_d = nc.declare_dram_parameter("xcol", [8, BL * L], BF16, isOutput=False)
    wd = {name: nc.declare_dram_parameter(name, shape, dt, isOutput=False)
          for name, shape, dt in _WSPEC}
    y_d = nc.declare_dram_parameter("y", [BL, 4, L2], F32, isOutput=True)
    dbg = {}
    if DEBUG_OUT:
        for nm, cols in (("dbgE", L), ("dbgH0F", L), ("dbgH0R", L),
                         ("dbgH1F", L + 2 * PAD), ("dbgH1R", L + 2 * PAD)):
            dbg[nm] = nc.declare_dram_parameter(nm, [HS, BL * cols], BF16,
                                                isOutput=True)

    with tile.TileContext(nc) as tc:
        with ExitStack() as top:
            wpool = top.enter_context(tc.tile_pool(name="w", bufs=1))
            state = top.enter_context(tc.tile_pool(name="state", bufs=1))
            psum_r = top.enter_context(tc.tile_pool(name="psum_r", bufs=2,
                                                    space="PSUM"))
            otile = top.enter_context(tc.tile_pool(name="otile", bufs=3))
            tmp = top.enter_context(tc.tile_pool(name="tmp", bufs=2))

            wt = {}
            for name, shape, dt in _WSPEC:
                wt[name] = wpool.tile(shape, dt, tag=f"w_{name}", name=f"w_{name}")
                nc.sync.dma_start(wt[name][:], wd[name][:])
            xcol = wpool.tile([8, BL * L], BF16, tag="xcol")
            nc.sync.dma_start(xcol[:], xcol_d[:])

            cg_tiles = []
            for si in range(NS):
                cg = state.tile([HS, SB, 2], F32, tag=f"cg{si}", name=f"cg{si}")
                nc.gpsimd.memset(cg[:], 0.0)
                cg_tiles.append(cg)
            hz = state.tile([HS, BL], BF16, tag="hz")
            nc.gpsimd.memset(hz[:], 0.0)

            pools = {"psum_r": psum_r, "ind": wt["ind"], "otile": otile,
                     "tmp": tmp}

            bigpool = top.enter_context(tc.tile_pool(name="big", bufs=1))
            dpool = top.enter_context(tc.tile_pool(name="dec", bufs=2))

            # ---- encoder ----
            E = bigpool.tile([HS, BL, L], BF16, tag="E")
            for b in range(BL):
                pe = psum_r.tile([HS, 512], F32, tag="pg0")
                nc.tensor.matmul(pe[:], wt["w1"][:], xcol[:, b * L:(b + 1) * L],
                                 start=True, stop=True)
                nc.scalar.activation(E[:, b, :], pe[:], AF.Relu,
                                     bias=wt["b1"][:], scale=wt["s1"][:])

            # ---- 4 LSTM passes ----
            def wdict(tag):
                d = {"U": [wt[f"U{tag}_{j}"] for j in range(4)],
                     "Wa": [wt[f"Wa{tag}_{j}"] for j in range(4)],
                     "bA": wt[f"bA{tag}"], "bB": wt[f"bB{tag}"]}
                if tag.startswith("1"):
                    d["Wb"] = [wt[f"Wb{tag}_{j}"] for j in range(4)]
                return d

            H0F = bigpool.tile([HS, BL, L], BF16, tag="H0F")
            H0R = bigpool.tile([HS, BL, L], BF16, tag="H0R")
            h_aps = [hz[:, si * SB:(si + 1) * SB] for si in range(NS)]
            h_aps = _emit_pass(nc, pools, wdict("0f"), [E[:]], H0F, h_aps,
                               cg_tiles, False, 0)
            h_aps = _emit_pass(nc, pools, wdict("0r"), [E[:]], H0R, h_aps,
                               cg_tiles, True, 0)

            LS = L + 2 * PAD
            H1F = bigpool.tile([HS, BL, LS], BF16, tag="H1F")
            H1R = bigpool.tile([HS, BL, LS], BF16, tag="H1R")
            for Hb in (H1F, H1R):
                nc.gpsimd.memset(Hb[:, :, 0:PAD], 0.0)
                nc.gpsimd.memset(Hb[:, :, PAD + L:LS], 0.0)
            h_aps = _emit_pass(nc, pools, wdict("1f"), [H0F[:], H0R[:]], H1F,
                               h_aps, cg_tiles, False, PAD)
            h_aps = _emit_pass(nc, pools, wdict("1r"), [H0F[:], H0R[:]], H1R,
                               h_aps, cg_tiles, True, PAD)

            if DEBUG_OUT:
                for nm, til in (("dbgE", E), ("dbgH0F", H0F), ("dbgH0R", H0R),
                                ("dbgH1F", H1F), ("dbgH1R", H1R)):
                    nc.sync.dma_start(
                        dbg[nm][:], til[:].rearrange("p b t -> p (b t)"))

            # ---- decoder: groups of 4 samples, stage-major ----
            S1 = L1 + 2 * PAD
            S2 = L2 + 2 * PAD

            def conv_chunks(dst, src, taps_w, n_len, scale, bias, out_parts,
                            base_part=0, ptag="pg0"):
                """K-tap conv with per-chunk psum; batch the act over both
                512-chunks of each 1024 window."""
                nchunks = n_len // 512
                for c0 in range(0, nchunks, 2):
                    nb = min(2, nchunks - c0)
                    pd = pools["psum_r"].tile([HS, 2, 512], F32, tag=ptag,
                                              name="pd")
                    for ci in range(nb):
                        n0 = (c0 + ci) * 512
                        for i, (delta, k) in enumerate(K5_TAPS):
                            rhs = src[:, PAD + n0 + delta: PAD + n0 + delta + 512]
                            nc.tensor.matmul(
                                pd[base_part:base_part + out_parts, ci, :],
                                taps_w[k][:, :out_parts], rhs,
                                start=(i == 0), stop=(i == 4))
                    nc.scalar.activation(
                        dst[base_part:base_part + out_parts,
                            PAD + c0 * 512: PAD + (c0 + nb) * 512],
                        pd[base_part:base_part + out_parts, 0:nb, :]
                        .rearrange("p c n -> p (c n)"),
                        AF.Relu,
                        bias=bias[base_part:base_part + out_parts],
                        scale=scale[base_part:base_part + out_parts])

            for g0 in range(0, BL, 4):
                D1s, D2s, D3s = [], [], []
                # up1: per sample
                for si in range(4):
                    b = g0 + si
                    D1 = dpool.tile([HS, S1], BF16, tag=f"D1_{si}", name="D1")
                    nc.gpsimd.memset(D1[:, 0:PAD], 0.0)
                    nc.gpsimd.memset(D1[:, PAD + L1:S1], 0.0)
                    dv = D1.rearrange("p (m r) -> p m r", r=2)
                    pd = psum_r.tile([HS, 2, 512], F32, tag="pg0", name="pd")
                    for r in (0, 1):
                        first = True
                        for ti, (delta, _) in enumerate(CONVT_TAPS[r]):
                            for ch, Hb in enumerate((H1F, H1R)):
                                blk = (r * 4 + ti) * 2 + ch
                                rhs = Hb[:, b, PAD + delta: PAD + delta + 512]
                                nc.tensor.matmul(
                                    pd[:, r, :], wt[f"u1_{blk}"][:], rhs,
                                    start=first, stop=(ti == 3 and ch == 1))
                                first = False
                    for r in (0, 1):
                        nc.scalar.activation(
                            dv[:, PAD // 2: PAD // 2 + 512, r],
                            pd[:, r, :], AF.Copy)
                    D1s.append(D1)
                # d11
                for si in range(4):
                    D2 = dpool.tile([HS, S1], BF16, tag=f"D2_{si}", name="D2")
                    nc.gpsimd.memset(D2[:, 0:PAD], 0.0)
                    nc.gpsimd.memset(D2[:, PAD + L1:S1], 0.0)
                    conv_chunks(D2, D1s[si], [wt[f"d11_{k}"] for k in range(5)],
                                L1, wt["s11"], wt["b11"], HS, ptag="pg1")
                    D2s.append(D2)
                # d12 -> paired tiles [128, S1]: rows 0:64 sample a, 64:128 b
                for pi in range(2):
                    D3 = dpool.tile([HS, S1], BF16, tag=f"D3_{pi}", name="D3")
                    nc.gpsimd.memset(D3[:, 0:PAD], 0.0)
                    nc.gpsimd.memset(D3[:, PAD + L1:S1], 0.0)
                    for si in (0, 1):
                        conv_chunks(D3, D2s[pi * 2 + si],
                                    [wt[f"d12_{k}"] for k in range(5)],
                                    L1, wt["s12"], wt["b12"], 64,
                                    base_part=64 * si, ptag="pg0")
                    D3s.append(D3)
                # up2 -> D4 [128, S2]: 4 samples x 32ch (2-sample block-diag)
                D4 = dpool.tile([HS, S2], BF16, tag="D4", name="D4")
                nc.gpsimd.memset(D4[:, 0:PAD], 0.0)
                nc.gpsimd.memset(D4[:, PAD + L2:S2], 0.0)
                dv4 = D4.rearrange("p (m r) -> p m r", r=2)
                for pi in range(2):
                    rows = slice(pi * 64, (pi + 1) * 64)
                    for r in (0, 1):
                        pd = psum_r.tile([HS, 2, 512], F32,
                                         tag=f"pg{r}", name="pd")
                        for m0 in (0, 1):
                            first = True
                            for ti, (delta, _) in enumerate(CONVT_TAPS[r]):
                                rhs = D3s[pi][:, PAD + m0 * 512 + delta:
                                              PAD + m0 * 512 + delta + 512]
                                nc.tensor.matmul(
                                    pd[rows, m0, :], wt[f"u2_{r}{ti}"][:],
                                    rhs, start=first, stop=(ti == 3))
                                first = False
                        nc.scalar.activation(
                            dv4[rows, PAD // 2: PAD // 2 + 1024, r],
                            pd[rows].rearrange("p m n -> p (m n)"), AF.Copy)
                # d21: 4-sample block-diag
                D5 = dpool.tile([HS, S2], BF16, tag="D5", name="D5")
                nc.gpsimd.memset(D5[:, 0:PAD], 0.0)
                nc.gpsimd.memset(D5[:, PAD + L2:S2], 0.0)
                conv_chunks(D5, D4, [wt[f"d21_{k}"] for k in range(5)],
                            L2, wt["s21"], wt["b21"], HS, ptag="pg0")
                # d22: 4-sample block-diag -> [16, 512] chunks -> DMA
                for c0 in range(0, 4, 2):
                    pd = psum_r.tile([HS, 2, 512], F32, tag="pg1", name="pd")
                    ot = otile.tile([16, 2, 512], F32, tag="ot", name="ot")
                    for ci in range(2):
                        n0 = (c0 + ci) * 512
                        for i, (delta, k) in enumerate(K5_TAPS):
                            rhs = D5[:, PAD + n0 + delta: PAD + n0 + delta + 512]
                            nc.tensor.matmul(pd[0:16, ci, :],
                                             wt[f"d22_{k}"][:], rhs,
                                             start=(i == 0), stop=(i == 4))
                    nc.scalar.activation(
                        ot[:].rearrange("p c n -> p (c n)"),
                        pd[0:16].rearrange("p c n -> p (c n)"),
                        AF.Relu, bias=wt["b22"][:], scale=wt["s22"][:])
                    # rows = (sample, class)
                    nc.sync.dma_start(
                        y_d[g0:g0 + 4, :, c0 * 512:(c0 + 2) * 512]
                        .rearrange("b c n -> (b c) n"),
                        ot[:].rearrange("p c n -> p (c n)"))
    nc.finalize()
    return nc


_NC = None


def _get_nc():
    global _NC
    if _NC is None:
        _NC = build_nc()
    return _NC


def kernel(**inputs):
    nc = _get_nc()
    w = _prep_weights(inputs)
    x = np.asarray(inputs["x"], np.float32)
    in_maps = []
    for c in range(N_CORES):
        m = dict(w)
        m["xcol"] = _prep_xcol(x[c * BL:(c + 1) * BL])
        in_maps.append(m)
    trace = bool(int(os.environ.get("BASS_KERNEL_TRACE", "0")))
    res = run_bass_kernel_spmd(nc, in_maps, list(range(N_CORES)), trace=trace)
    if trace:
        kernel.last_exec_time_ns = res.exec_time_ns
    out = np.concatenate([res.results[i]["y"] for i in range(N_CORES)], axis=0)
    return np.ascontiguousarray(out.astype(np.float32))


# revision 28
# speedup vs baseline: 1.2499x; 1.2499x over previous
"""Trainium2 Bass kernel for CNN-BiLSTM encoder/decoder (nn_CNN_BiLSTM_AttenQ).

Data-parallel over batch: B=128 sharded 8 ways (16 samples/core), weights
replicated, no collectives.

v2 design (vs baseline):
  recurrence: per stream-step cell is 1 ACT + 2 DVE + 1 add (gpsimd/dve):
    PSUM gates per step [f|i|g|o] (f,i,o pre-scaled by 0.25, +0.5 bias so
    clamp01 == hard-sigmoid). ACT relu-copies g into CG odd slots; one
    fused DVE op (FCM) computes (min(relu(f),1)*c, min(relu(i),1)*g)
    in-place into CG; add folds CG even+odd -> c'; HSIG_MUL produces h.
  U/W gate blocks are separate contiguous [128,128] tiles (FWL weight loads),
  one weight load shared by both batch streams.
  decoder: stage-major over groups of 4 samples; 32-channel convs packed
  block-diagonally (d21/d22: 4 samples, up2: 2 samples) to fill the PE
  array; PSUM->SBUF copies batched into single wide ACT ops.
"""

import os
import numpy as np
import ml_dtypes

import bass_rust
import concourse.mybir as mybir
import concourse.tile as tile
from concourse import bacc
from concourse import dve_ops
from concourse.dve_spec import (Spec, Src0, Src1, C0, C1, One, relu, minn,
                                lower, _has_src1)
from concourse.dve_uop import DveOpSpec
from concourse.bass_utils import run_bass_kernel_spmd
from contextlib import ExitStack


def _register_dve_op(name, body, ref):
    for op in dve_ops.OPS:
        if op.name == name:
            return op
    spec = Spec(body=body, reference=ref)
    op = dve_ops.DveOp(name, spec, subdim=False, uops_sha={})
    dve_ops._SUB_OPCODE_FOR_NAME[name] = max(dve_ops._SUB_OPCODE_FOR_NAME.values()) + 1
    dve_ops.OPS.append(op)
    dve_ops.CUSTOM_DVE_SPECS[name] = spec
    for ver in ("v3", "v4"):
        uops = lower(spec, ver=ver)
        op.uops_sha[ver] = DveOpSpec(
            name=name, opcode=dve_ops.get_dve_sub_opcode(name), uops=uops,
            rd1_en=_has_src1(spec)).sha(ver)
    return op


# out = min(relu(in0),1) * relu(in1)   (pairs (f,c),(i,g))
FCM = _register_dve_op(
    "ANT_FCM", minn(relu(Src0), One) * relu(Src1),
    lambda in0, in1, s0, s1, imm2: np.minimum(np.maximum(in0, 0), 1)
    * np.maximum(in1, 0))
# out = clamp01(in0*s0 + s1) * clamp01(in1)
HSIG_MUL = _register_dve_op(
    "ANT_HSIG_MUL",
    minn(relu(Src0 * C0 + C1), One) * minn(relu(Src1), One),
    lambda in0, in1, s0, s1, imm2: np.minimum(np.maximum(in0 * s0 + s1, 0), 1)
    * np.minimum(np.maximum(in1, 0), 1))

F32 = mybir.dt.float32
BF16 = mybir.dt.bfloat16
AF = mybir.ActivationFunctionType
OP = mybir.AluOpType
BFNP = ml_dtypes.bfloat16

B, T, C, HS = 128, 2048, 128, 128
N_CORES = 8
BL = B // N_CORES          # 16 samples per core
L = T // 4                 # 512 encoder output length
CH = 32                    # recurrence chunk length (timesteps)
NCH = L // CH
L1 = 2 * L                 # 1024
L2 = T                     # 2048
PAD = 4                    # halo pad per batch segment in decoder buffers
EPS = 1e-5
NS = 2                     # phase-offset streams per core
SB = BL // NS              # 8 samples per stream

# convT taps: out[2m+r] += x[m+delta] @ w[:, :, k]  -> list of (delta, k)
CONVT_TAPS = {0: [(1, 1), (0, 3), (-1, 5), (-2, 7)],
              1: [(2, 0), (1, 2), (0, 4), (-1, 6)]}
K5_TAPS = [(k - 2, k) for k in range(5)]

ADD_ENGINE = os.environ.get("BASS_ADD_ENGINE", "dve")


def _bf(x):
    return np.ascontiguousarray(np.asarray(x, np.float32).astype(BFNP))


def _f32(x):
    return np.ascontiguousarray(np.asarray(x, np.float32))


def _prep_dir(W, U, b):
    # torch gate order [i,f,g,o] -> v2 order [f,i,g,o]
    perm = [1, 0, 2, 3]
    Wp = np.concatenate([W[:, j * HS:(j + 1) * HS] for j in perm], 1).astype(np.float64)
    Up = np.concatenate([U[:, j * HS:(j + 1) * HS] for j in perm], 1).astype(np.float64)
    bp = np.concatenate([b[j * HS:(j + 1) * HS] for j in perm]).astype(np.float64)
    # f,i,o scaled 0.25 (+0.5 bias); g raw (relu)
    for j in (0, 1, 3):
        Wp[:, j * HS:(j + 1) * HS] *= 0.25
        Up[:, j * HS:(j + 1) * HS] *= 0.25
        bp[j * HS:(j + 1) * HS] = 0.25 * bp[j * HS:(j + 1) * HS] + 0.5
    bA = np.stack([bp[0:HS], bp[HS:2 * HS]])          # (b_f, b_i)
    bB = np.stack([bp[2 * HS:3 * HS], bp[3 * HS:]])   # (b_g, b_o)
    return Wp, Up, _bf(bA), _bf(bB)


def _prep_weights(inp):
    w = {}
    w["w1"] = _bf(np.asarray(inp["conv1_w"])[:, 0, :].T)            # [8,128]
    w["s1"] = _f32(np.asarray(inp["bn1_g"]) / np.sqrt(1.0 + EPS))[:, None]
    w["b1"] = _f32(np.asarray(inp["bn1_b"]))[:, None]
    for tag, (Wk, Uk, bk) in {
        "0f": ("W0f", "U0f", "b0f"), "0r": ("W0r", "U0r", "b0r"),
        "1f": ("W1f", "U1f", "b1f"), "1r": ("W1r", "U1r", "b1r"),
    }.items():
        Wp, Up, bA, bB = _prep_dir(np.asarray(inp[Wk], np.float32),
                                   np.asarray(inp[Uk], np.float32),
                                   np.asarray(inp[bk], np.float32))
        for j in range(4):
            w[f"U{tag}_{j}"] = _bf(Up[:, j * HS:(j + 1) * HS])
            w[f"Wa{tag}_{j}"] = _bf(Wp[:HS, j * HS:(j + 1) * HS])
            if Wp.shape[0] == 2 * HS:
                w[f"Wb{tag}_{j}"] = _bf(Wp[HS:, j * HS:(j + 1) * HS])
        w[f"bA{tag}"], w[f"bB{tag}"] = bA, bB
    # up1: 16 blocks [128,128]: blk = (r*4+ti)*2+ch
    u1w = np.asarray(inp["up1_w"], np.float32)                      # [256,128,8]
    for r in (0, 1):
        for ti, (_, k) in enumerate(CONVT_TAPS[r]):
            for ch in (0, 1):
                blk = (r * 4 + ti) * 2 + ch
                w[f"u1_{blk}"] = _bf(u1w[ch * HS:(ch + 1) * HS, :, k])
    d11w = np.asarray(inp["d11_w"], np.float32)
    for k in range(5):
        w[f"d11_{k}"] = _bf(d11w[:, :, k].T)
    w["s11"] = _f32(np.asarray(inp["bn11_g"]) / np.sqrt(1.0 + EPS))[:, None]
    w["b11"] = _f32(np.asarray(inp["bn11_b"]))[:, None]
    d12w = np.asarray(inp["d12_w"], np.float32)
    for k in range(5):
        w[f"d12_{k}"] = _bf(d12w[:, :, k].T)
    # replicated x2 so partition-base-64 slices stay aligned
    w["s12"] = _f32(np.tile(np.asarray(inp["bn12_g"]) / np.sqrt(1.0 + EPS), 2))[:, None]
    w["b12"] = _f32(np.tile(np.asarray(inp["bn12_b"]), 2))[:, None]
    # up2: block-diag 2-sample [128, 64] per (r, ti)
    u2w = np.asarray(inp["up2_w"], np.float32)                      # [64,32,8]
    for r in (0, 1):
        for ti, (_, k) in enumerate(CONVT_TAPS[r]):
            bd = np.zeros((128, 64), np.float32)
            for s in (0, 1):
                bd[s * 64:(s + 1) * 64, s * 32:(s + 1) * 32] = u2w[:, :, k]
            w[f"u2_{r}{ti}"] = _bf(bd)
    # d21: block-diag 4-sample [128,128] per tap
    d21w = np.asarray(inp["d21_w"], np.float32)
    for k in range(5):
        bd = np.zeros((128, 128), np.float32)
        for s in range(4):
            bd[s * 32:(s + 1) * 32, s * 32:(s + 1) * 32] = d21w[:, :, k].T
        w[f"d21_{k}"] = _bf(bd)
    w["s21"] = _f32(np.tile(np.asarray(inp["bn21_g"]) / np.sqrt(1.0 + EPS), 4))[:, None]
    w["b21"] = _f32(np.tile(np.asarray(inp["bn21_b"]), 4))[:, None]
    # d22: block-diag 4-sample [128,16] per tap
    d22w = np.asarray(inp["d22_w"], np.float32)
    for k in range(5):
        bd = np.zeros((128, 16), np.float32)
        for s in range(4):
            bd[s * 32:(s + 1) * 32, s * 4:(s + 1) * 4] = d22w[:, :, k].T
        w[f"d22_{k}"] = _bf(bd)
    w["s22"] = _f32(np.tile(np.asarray(inp["bn22_g"]) / np.sqrt(1.0 + EPS), 4))[:, None]
    w["b22"] = _f32(np.tile(np.asarray(inp["bn22_b"]), 4))[:, None]
    # bias-spread indicator [2, 512]: gate-major: row0 on first 256 cols
    blk = CH * SB
    ind = np.zeros((2, 2 * blk), np.float32)
    ind[0, :blk] = 1.0
    ind[1, blk:] = 1.0
    w["ind"] = _bf(ind)
    return w


def _prep_xcol(x_shard):
    xp = np.pad(np.asarray(x_shard, np.float32)[:, 0, :], ((0, 0), (3, 4)))
    cols = np.stack([xp[:, k:k + 4 * L:4] for k in range(8)])       # [8,BL,L]
    return _bf(cols.reshape(8, BL * L))


def _wspec():
    spec = [
        ("w1", [8, HS], BF16), ("s1", [HS, 1], F32), ("b1", [HS, 1], F32),
        ("ind", [2, CH * 2 * SB], BF16),
    ]
    for tag in ("0f", "0r", "1f", "1r"):
        for j in range(4):
            spec.append((f"U{tag}_{j}", [HS, HS], BF16))
            spec.append((f"Wa{tag}_{j}", [HS, HS], BF16))
            if tag.startswith("1"):
                spec.append((f"Wb{tag}_{j}", [HS, HS], BF16))
        spec.append((f"bA{tag}", [2, HS], BF16))
        spec.append((f"bB{tag}", [2, HS], BF16))
    for blk in range(16):
        spec.append((f"u1_{blk}", [HS, HS], BF16))
    for k in range(5):
        spec.append((f"d11_{k}", [HS, HS], BF16))
        spec.append((f"d12_{k}", [HS, 64], BF16))
        spec.append((f"d21_{k}", [HS, HS], BF16))
        spec.append((f"d22_{k}", [HS, 16], BF16))
    for r in (0, 1):
        for ti in range(4):
            spec.append((f"u2_{r}{ti}", [HS, 64], BF16))
    spec += [("s11", [HS, 1], F32), ("b11", [HS, 1], F32),
             ("s12", [HS, 1], F32), ("b12", [HS, 1], F32),
             ("s21", [HS, 1], F32), ("b21", [HS, 1], F32),
             ("s22", [16, 1], F32), ("b22", [16, 1], F32)]
    return spec


_WSPEC = _wspec()


def _emit_pass(nc, pools, wt, ins, Hout, h_aps, cg_tiles, reverse, t_off):
    """One LSTM direction (L steps, NCH chunks), NS streams.
    ins: input APs [128,BL,L]. Hout: [128,BL,Lseg]; h at [:,bs,t_off+t].
    cg_tiles[si]: [128, 8, 2] view-able CG state (even=c, odd=g-scratch)."""
    psum_pool = pools["psum_r"]
    ind = pools["ind"]
    h_aps = list(h_aps)

    def chunk_pieces(ci, si):
        """Allocate pg for chunk ci of stream si; return (pg, T0, pieces).
        pg gate-major [HS, 4(gates f,i,g,o), CH, SB]: every matmul out is a
        contiguous single-bank region."""
        T0 = ci * CH if not reverse else L - (ci + 1) * CH
        pg = psum_pool.tile([HS, 4, CH, SB], F32, tag=f"pg{si}", name=f"pg{si}")

        def bias():
            nc.tensor.matmul(pg[:, 0:2, :, :], wt["bA"][:], ind[:],
                             start=True, stop=False)
            nc.tensor.matmul(pg[:, 2:4, :, :], wt["bB"][:], ind[:],
                             start=True, stop=False)

        pieces = [bias]
        for j in range(4):
            def wmm(j=j):
                Wlists = [wt["Wa"]] + ([wt["Wb"]] if "Wb" in wt else [])
                for idx, Wl in enumerate(Wlists):
                    rhs = ins[idx].rearrange("p b t -> p t b")[
                        :, T0:T0 + CH, si * SB:(si + 1) * SB]
                    nc.tensor.matmul(pg[:, j, :, :], Wl[j][:], rhs,
                                     start=False, stop=False)
            pieces.append(wmm)
        return pg, T0, pieces

    nxt = [chunk_pieces(0, si) for si in range(NS)]
    for _, _, pieces in nxt:
        for p in pieces:
            p()
    for ci in range(NCH):
        pgs = [pg for pg, _, _ in nxt]
        T0 = nxt[0][1]
        if ci + 1 < NCH:
            nxt = [chunk_pieces(ci + 1, si) for si in range(NS)]
            todo = [p for _, _, pieces in nxt for p in pieces]
        else:
            nxt, todo = [], []
        n_todo, done = len(todo), 0
        for s in range(CH):
            tl = s if not reverse else CH - 1 - s
            t = T0 + tl
            # U matmuls per stream, g-gate first: the ACT g-copy (the head
            # of the serial DVE cell chain) unblocks ~2 MM-slots earlier
            for si in range(NS):
                for j in (2, 0, 1, 3):
                    nc.tensor.matmul(pgs[si][:, j, tl, :],
                                     wt["U"][j][:], h_aps[si],
                                     start=False, stop=(j == 3))
            for si in range(NS):
                pg, cgq = pgs[si], cg_tiles[si]
                # g -> CG odd (relu)
                nc.scalar.activation(cgq[:, :, 1], pg[:, 2, tl, :], AF.Relu)
                # (f,c),(i,g) -> T = (f^*c, i^*relu(g))
                tt = pools["tmp"].tile([HS, SB, 2], F32, tag=f"t{si}", name="tt")
                in0 = pg[:, 0:2, tl, :].rearrange("p q s -> p s q")
                nc.vector._custom_dve(FCM, out=tt[:], in0=in0, in1=cgq[:])
                # c' = cf + t1
                if ADD_ENGINE == "gp":
                    nc.gpsimd.tensor_tensor(cgq[:, :, 0], tt[:, :, 0],
                                            tt[:, :, 1], OP.add)
                else:
                    nc.vector.tensor_tensor(cgq[:, :, 0], tt[:, :, 0],
                                            tt[:, :, 1], OP.add)
                # h = clamp01(0.25c+0.5)*clamp01(o)
                h_aps[si] = Hout[:, si * SB:(si + 1) * SB, t_off + t]
                nc.vector._custom_dve(HSIG_MUL, out=h_aps[si],
                                      in0=cgq[:, :, 0], in1=pg[:, 3, tl, :],
                                      s0=0.25, s1=0.5)
            # spread next-chunk precompute pieces evenly across steps
            while todo and done < (s + 1) * n_todo // CH:
                todo.pop(0)()
                done += 1
    return h_aps


DEBUG_OUT = bool(int(os.environ.get("BASS_DEBUG_OUT", "0")))


def build_nc():
    nc = bacc.Bacc()
    xcol_d = nc.declare_dram_parameter("xcol", [8, BL * L], BF16, isOutput=False)
    wd = {name: nc.declare_dram_parameter(name, shape, dt, isOutput=False)
          for name, shape, dt in _WSPEC}
    y_d = nc.declare_dram_parameter("y", [BL, 4, L2], F32, isOutput=True)
    dbg = {}
    if DEBUG_OUT:
        for nm, cols in (("dbgE", L), ("dbgH0F", L), ("dbgH0R", L),
                         ("dbgH1F", L + 2 * PAD), ("dbgH1R", L + 2 * PAD)):
            dbg[nm] = nc.declare_dram_parameter(nm, [HS, BL * cols], BF16,
                                                isOutput=True)

    with tile.TileContext(nc) as tc:
        with ExitStack() as top:
            wpool = top.enter_context(tc.tile_pool(name="w", bufs=1))
            state = top.enter_context(tc.tile_pool(name="state", bufs=1))
            psum_r = top.enter_context(tc.tile_pool(name="psum_r", bufs=2,
                                                    space="PSUM"))
            otile = top.enter_context(tc.tile_pool(name="otile", bufs=3))
            tmp = top.enter_context(tc.tile_pool(name="tmp", bufs=2))

            wt = {}
            for name, shape, dt in _WSPEC:
                wt[name] = wpool.tile(shape, dt, tag=f"w_{name}", name=f"w_{name}")
                nc.sync.dma_start(wt[name][:], wd[name][:])
            xcol = wpool.tile([8, BL * L], BF16, tag="xcol")
            nc.sync.dma_start(xcol[:], xcol_d[:])

            cg_tiles = []
            for si in range(NS):
                cg = state.tile([HS, SB, 2], F32, tag=f"cg{si}", name=f"cg{si}")
                nc.gpsimd.memset(cg[:], 0.0)
                cg_tiles.append(cg)
            hz = state.tile([HS, BL], BF16, tag="hz")
            nc.gpsimd.memset(hz[:], 0.0)

            pools = {"psum_r": psum_r, "ind": wt["ind"], "otile": otile,
                     "tmp": tmp}

            bigpool = top.enter_context(tc.tile_pool(name="big", bufs=1))
            dpool = top.enter_context(tc.tile_pool(name="dec", bufs=2))

            # ---- encoder ----
            E = bigpool.tile([HS, BL, L], BF16, tag="E")
            for b in range(BL):
                pe = psum_r.tile([HS, 512], F32, tag="pg0")
                nc.tensor.matmul(pe[:], wt["w1"][:], xcol[:, b * L:(b + 1) * L],
                                 start=True, stop=True)
                nc.scalar.activation(E[:, b, :], pe[:], AF.Relu,
                                     bias=wt["b1"][:], scale=wt["s1"][:])

            # ---- 4 LSTM passes ----
            def wdict(tag):
                d = {"U": [wt[f"U{tag}_{j}"] for j in range(4)],
                     "Wa": [wt[f"Wa{tag}_{j}"] for j in range(4)],
                     "bA": wt[f"bA{tag}"], "bB": wt[f"bB{tag}"]}
                if tag.startswith("1"):
                    d["Wb"] = [wt[f"Wb{tag}_{j}"] for j in range(4)]
                return d

            H0F = bigpool.tile([HS, BL, L], BF16, tag="H0F")
            H0R = bigpool.tile([HS, BL, L], BF16, tag="H0R")
            h_aps = [hz[:, si * SB:(si + 1) * SB] for si in range(NS)]
            h_aps = _emit_pass(nc, pools, wdict("0f"), [E[:]], H0F, h_aps,
                               cg_tiles, False, 0)
            h_aps = _emit_pass(nc, pools, wdict("0r"), [E[:]], H0R, h_aps,
                               cg_tiles, True, 0)

            LS = L + 2 * PAD
            H1F = bigpool.tile([HS, BL, LS], BF16, tag="H1F")
            H1R = bigpool.tile([HS, BL, LS], BF16, tag="H1R")
            for Hb in (H1F, H1R):
                nc.gpsimd.memset(Hb[:, :, 0:PAD], 0.0)
                nc.gpsimd.memset(Hb[:, :, PAD + L:LS], 0.0)
            h_aps = _emit_pass(nc, pools, wdict("1f"), [H0F[:], H0R[:]], H1F,
                               h_aps, cg_tiles, False, PAD)
            h_aps = _emit_pass(nc, pools, wdict("1r"), [H0F[:], H0R[:]], H1R,
                               h_aps, cg_tiles, True, PAD)

            if DEBUG_OUT:
                for nm, til in (("dbgE", E), ("dbgH0F", H0F), ("dbgH0R", H0R),
                                ("dbgH1F", H1F), ("dbgH1R", H1R)):
                    nc.sync.dma_start(
                        dbg[nm][:], til[:].rearrange("p b t -> p (b t)"))

            # ---- decoder: groups of 4 samples, stage-major ----
            S1 = L1 + 2 * PAD
            S2 = L2 + 2 * PAD

            def conv_chunks(dst, src, taps_w, n_len, scale, bias, out_parts,
                            base_part=0, ptag="pg0"):
                """K-tap conv with per-chunk psum; batch the act over both
                512-chunks of each 1024 window."""
                nchunks = n_len // 512
                for c0 in range(0, nchunks, 2):
                    nb = min(2, nchunks - c0)
                    pd = pools["psum_r"].tile([HS, 2, 512], F32, tag=ptag,
                                              name="pd")
                    for ci in range(nb):
                        n0 = (c0 + ci) * 512
                        for i, (delta, k) in enumerate(K5_TAPS):
                            rhs = src[:, PAD + n0 + delta: PAD + n0 + delta + 512]
                            nc.tensor.matmul(
                                pd[base_part:base_part + out_parts, ci, :],
                                taps_w[k][:, :out_parts], rhs,
                                start=(i == 0), stop=(i == 4))
                    nc.scalar.activation(
                        dst[base_part:base_part + out_parts,
                            PAD + c0 * 512: PAD + (c0 + nb) * 512],
                        pd[base_part:base_part + out_parts, 0:nb, :]
                        .rearrange("p c n -> p (c n)"),
                        AF.Relu,
                        bias=bias[base_part:base_part + out_parts],
                        scale=scale[base_part:base_part + out_parts])

            for g0 in range(0, BL, 4):
                D1s, D2s, D3s = [], [], []
                # up1: per sample
                for si in range(4):
                    b = g0 + si
                    D1 = dpool.tile([HS, S1], BF16, tag=f"D1_{si}", name="D1")
                    nc.gpsimd.memset(D1[:, 0:PAD], 0.0)
                    nc.gpsimd.memset(D1[:, PAD + L1:S1], 0.0)
                    dv = D1.rearrange("p (m r) -> p m r", r=2)
                    pd = psum_r.tile([HS, 2, 512], F32, tag="pg0", name="pd")
                    for r in (0, 1):
                        first = True
                        for ti, (delta, _) in enumerate(CONVT_TAPS[r]):
                            for ch, Hb in enumerate((H1F, H1R)):
                                blk = (r * 4 + ti) * 2 + ch
                                rhs = Hb[:, b, PAD + delta: PAD + delta + 512]
                                nc.tensor.matmul(
                                    pd[:, r, :], wt[f"u1_{blk}"][:], rhs,
                                    start=first, stop=(ti == 3 and ch == 1))
                                first = False
                    for r in (0, 1):
                        nc.scalar.activation(
                            dv[:, PAD // 2: PAD // 2 + 512, r],
                            pd[:, r, :], AF.Copy)
                    D1s.append(D1)
                # d11
                for si in range(4):
                    D2 = dpool.tile([HS, S1], BF16, tag=f"D2_{si}", name="D2")
                    nc.gpsimd.memset(D2[:, 0:PAD], 0.0)
                    nc.gpsimd.memset(D2[:, PAD + L1:S1], 0.0)
                    conv_chunks(D2, D1s[si], [wt[f"d11_{k}"] for k in range(5)],
                                L1, wt["s11"], wt["b11"], HS, ptag="pg1")
                    D2s.append(D2)
                # d12 -> paired tiles [128, S1]: rows 0:64 sample a, 64:128 b
                for pi in range(2):
                    D3 = dpool.tile([HS, S1], BF16, tag=f"D3_{pi}", name="D3")
                    nc.gpsimd.memset(D3[:, 0:PAD], 0.0)
                    nc.gpsimd.memset(D3[:, PAD + L1:S1], 0.0)
                    for si in (0, 1):
                        conv_chunks(D3, D2s[pi * 2 + si],
                                    [wt[f"d12_{k}"] for k in range(5)],
                                    L1, wt["s12"], wt["b12"], 64,
                                    base_part=64 * si, ptag="pg0")
                    D3s.append(D3)
                # up2 -> D4 [128, S2]: 4 samples x 32ch (2-sample block-diag)
                D4 = dpool.tile([HS, S2], BF16, tag="D4", name="D4")
                nc.gpsimd.memset(D4[:, 0:PAD], 0.0)
                nc.gpsimd.memset(D4[:, PAD + L2:S2], 0.0)
                dv4 = D4.rearrange("p (m r) -> p m r", r=2)
                for pi in range(2):
                    rows = slice(pi * 64, (pi + 1) * 64)
                    for r in (0, 1):
                        pd = psum_r.tile([HS, 2, 512], F32,
                                         tag=f"pg{r}", name="pd")
                        for m0 in (0, 1):
                            first = True
                            for ti, (delta, _) in enumerate(CONVT_TAPS[r]):
                                rhs = D3s[pi][:, PAD + m0 * 512 + delta:
                                              PAD + m0 * 512 + delta + 512]
                                nc.tensor.matmul(
                                    pd[rows, m0, :], wt[f"u2_{r}{ti}"][:],
                                    rhs, start=first, stop=(ti == 3))
                                first = False
                        nc.scalar.activation(
                            dv4[rows, PAD // 2: PAD // 2 + 1024, r],
                            pd[rows].rearrange("p m n -> p (m n)"), AF.Copy)
                # d21: 4-sample block-diag
                D5 = dpool.tile([HS, S2], BF16, tag="D5", name="D5")
                nc.gpsimd.memset(D5[:, 0:PAD], 0.0)
                nc.gpsimd.memset(D5[:, PAD + L2:S2], 0.0)
                conv_chunks(D5, D4, [wt[f"d21_{k}"] for k in range(5)],
                            L2, wt["s21"], wt["b21"], HS, ptag="pg0")
                # d22: 4-sample block-diag -> [16, 512] chunks -> DMA
                for c0 in range(0, 4, 2):
                    pd = psum_r.tile([HS, 2, 512], F32, tag="pg1", name="pd")
                    ot = otile.tile([16, 2, 512], F32, tag="ot", name="ot")
                    for ci in range(2):
                        n0 = (c0 + ci) * 512
                        for i, (delta, k) in enumerate(K5_TAPS):
                            rhs = D5[:, PAD + n0 + delta: PAD + n0 + delta + 512]
                            nc.tensor.matmul(pd[0:16, ci, :],
                                             wt[f"d22_{k}"][:], rhs,
                                             start=(i == 0), stop=(i == 4))
                    nc.scalar.activation(
                        ot[:].rearrange("p c n -> p (c n)"),
                        pd[0:16].rearrange("p c n -> p (c n)"),
                        AF.Relu, bias=wt["b22"][:], scale=wt["s22"][:])
                    # rows = (sample, class)
                    nc.sync.dma_start(
                        y_d[g0:g0 + 4, :, c0 * 512:(c0 + 2) * 512]
                        .rearrange("b c n -> (b c) n"),
                        ot[:].rearrange("p c n -> p (c n)"))
    nc.finalize()
    return nc


_NC = None


def _get_nc():
    global _NC
    if _NC is None:
        _NC = build_nc()
    return _NC


def kernel(**inputs):
    nc = _get_nc()
    w = _prep_weights(inputs)
    x = np.asarray(inputs["x"], np.float32)
    in_maps = []
    for c in range(N_CORES):
        m = dict(w)
        m["xcol"] = _prep_xcol(x[c * BL:(c + 1) * BL])
        in_maps.append(m)
    trace = bool(int(os.environ.get("BASS_KERNEL_TRACE", "0")))
    res = run_bass_kernel_spmd(nc, in_maps, list(range(N_CORES)), trace=trace)
    if trace:
        kernel.last_exec_time_ns = res.exec_time_ns
    out = np.concatenate([res.results[i]["y"] for i in range(N_CORES)], axis=0)
    return np.ascontiguousarray(out.astype(np.float32))
